# revision 10
# baseline (speedup 1.0000x reference)
"""Trainium2 Bass kernel for nn_LinearEncoder (gnn_message_passing).

Reference computes, for N=512 nodes with n_in = n_out = 256:
    i, j = triu_indices(N, k=1)
    edges = concat([x[i], x[j]], -1)            # [E, 512]
    h = edges @ W.T + b                         # [E, 256]
    out[i, j] = h ; out = out + out.T           # [N, N, 256], 0 diagonal

Key algebraic identity: with W = [W1 | W2],
    h(i, j) = A[i] + B[j] + b,   A = x @ W1.T,  B = x @ W2.T
so the full output is
    out[i, j] = A[min(i,j)] + B'[max(i,j)]      (B' = B + b), 0 on diagonal.

The kernel shards output rows across 8 cores (64 rows each).  To keep a
single SPMD program, core k receives x pre-rotated by its row base
(x_rot[t] = x[(base+t) % 512]), which puts the triangular "diagonal block"
at local columns s in [0, 64) for every core.  Column/row table selection
(A vs B') differs per core only through small 0/1 mask *inputs*.  All the
broadcast-adds run on the TensorEngine as PSUM-accumulated matmuls:

    psum  = I128.T   @ ColTable          (copies the column table)
    psum += masks.T  @ row-table-slice   (adds the broadcast row term)

and the triangular diagonal block (including its zero diagonal) comes from
two constant masked-selection matmuls.  ScalarE/VectorE only evacuate PSUM
to SBUF; DMA streams ~33.5 MB/core of output back to HBM (the roofline).
"""

import os
import sys

for _p in ("/opt/trn_rl_repo", "/root/.axon_site/_ro/trn_rl_repo"):
    if os.path.isdir(_p) and _p not in sys.path:
        sys.path.insert(0, _p)

import numpy as np
import ml_dtypes

import concourse.bass as bass
import concourse.bacc as bacc
import concourse.mybir as mybir
import concourse.tile as tile
from concourse.bass_utils import run_bass_kernel_spmd

N = 512
CH = 256          # n_out
NIN = 256         # n_in
NCORES = 8
RB = N // NCORES  # 64 rows per core
F32 = mybir.dt.float32
F32R = mybir.dt.float32r
BF16 = mybir.dt.bfloat16
BF16NP = ml_dtypes.bfloat16

MODE = os.environ.get("LK_MODE", "bf16hl")  # "f32" | "f32r" | "bf16hl"


# --------------------------------------------------------------------------
# host-side constant builders
# --------------------------------------------------------------------------

def _masks_RL(k: int):
    """R/L region indicators over local columns s for core k."""
    base = RB * k
    wrap = N - base  # columns s >= wrap hold wrapped (j < base) entries
    s = np.arange(N)
    R = ((s >= 64) & (s < wrap)).astype(np.float32)
    L = (s >= wrap).astype(np.float32)
    return R, L


def _diag_consts():
    """Constant masked-selection weights for the 64x64 diagonal blocks.

    For row-pair rp, output column m = q*64 + s (q in {0,1}, s in [0,64)),
    with r_q = 2*rp + q and rhs = [A_rot[0:64] ; B'_rot[0:64]] (K = 128):
      L side (s < r_q):  value = A_rot[s] + B'_rot[r_q]
      R side (s > r_q):  value = B'_rot[s] + A_rot[r_q]
      s == r_q: all weights zero -> exact 0 output.
    """
    dl = np.zeros((128, 32 * 128), np.float32)
    dr = np.zeros((128, 32 * 128), np.float32)
    for rp in range(32):
        for q in range(2):
            r_q = 2 * rp + q
            for s in range(64):
                m = rp * 128 + q * 64 + s
                if s < r_q:
                    dl[s, m] = 1.0            # A_rot[s]
                    dl[64 + r_q, m] = 1.0     # B'_rot[r_q]
                elif s > r_q:
                    dr[64 + s, m] = 1.0       # B'_rot[s]
                    dr[r_q, m] = 1.0          # A_rot[r_q]
    return dl, dr


def _shared_inputs(W: np.ndarray, b: np.ndarray, mode: str):
    W = np.asarray(W, np.float32)
    b = np.asarray(b, np.float32)
    dl, dr = _diag_consts()
    i128 = np.eye(128, dtype=np.float32)
    i64p = np.concatenate([np.eye(64, dtype=np.float32)] * 2, axis=1)  # [64,128]
    out = {
        "w1t": np.ascontiguousarray(W[:, :NIN].T),   # [in, out]
        "w2t": np.ascontiguousarray(W[:, NIN:].T),
        "b_row": b.reshape(1, CH).copy(),
    }
    if mode == "bf16hl":
        cast = lambda a: a.astype(BF16NP)
    else:
        cast = lambda a: a.astype(np.float32)
    out["diag_l"] = cast(dl)
    out["diag_r"] = cast(dr)
    out["i128"] = cast(i128)
    out["i64p"] = cast(i64p)
    return out


def _core_inputs(x: np.ndarray, k: int, mode: str):
    x = np.asarray(x, np.float32)
    base = RB * k
    x_rot = np.roll(x, -base, axis=0)
    R, L = _masks_RL(k)

    cm = np.zeros((128, 8), np.float32)
    for t in range(4):
        cm[:, t] = R[128 * t:128 * (t + 1)]
        cm[:, 4 + t] = L[128 * t:128 * (t + 1)]

    if mode == "bf16hl":
        wm = np.stack([R, L, R, L]).astype(BF16NP)          # [4, 512]
        wm0 = np.zeros((8, 128), np.float32)
        p = np.arange(64)
        wm0[0, :64] = R[64 + p]
        wm0[1, :64] = L[64 + p]
        wm0[2, :64] = R[64 + p]
        wm0[3, :64] = L[64 + p]
        wm0[4, 64:] = R[64 + p]
        wm0[5, 64:] = L[64 + p]
        wm0[6, 64:] = R[64 + p]
        wm0[7, 64:] = L[64 + p]
        wm0 = wm0.astype(BF16NP)
    else:
        wm = np.stack([R, L]).astype(np.float32)            # [2, 512]
        wm0 = np.zeros((4, 128), np.float32)
        p = np.arange(64)
        wm0[0, :64] = R[64 + p]
        wm0[1, :64] = L[64 + p]
        wm0[2, 64:] = R[64 + p]
        wm0[3, 64:] = L[64 + p]
    return {
        "xt_rot": np.ascontiguousarray(x_rot.T),  # [in=256, node=512]
        "cm": cm,
        "wm": wm,
        "wm0": wm0,
    }


# --------------------------------------------------------------------------
# device program
# --------------------------------------------------------------------------

_PROGRAMS: dict = {}


def _build_program(mode: str) -> bass.Bass:
    nc = bacc.Bacc()
    f32 = F32
    mdt = {"f32": F32, "f32r": F32, "bf16hl": BF16}[mode]
    npad = 68  # padded flat scratch rows

    # ---- dram tensors -----------------------------------------------------
    xt_rot = nc.dram_tensor("xt_rot", [NIN, N], f32, kind="ExternalInput")
    w1t = nc.dram_tensor("w1t", [NIN, CH], f32, kind="ExternalInput")
    w2t = nc.dram_tensor("w2t", [NIN, CH], f32, kind="ExternalInput")
    b_row = nc.dram_tensor("b_row", [1, CH], f32, kind="ExternalInput")
    cm = nc.dram_tensor("cm", [128, 8], f32, kind="ExternalInput")
    d_wm = nc.dram_tensor("wm", [4 if mode == "bf16hl" else 2, N], mdt,
                          kind="ExternalInput")
    d_wm0 = nc.dram_tensor("wm0", [8 if mode == "bf16hl" else 4, 128], mdt,
                           kind="ExternalInput")
    d_dl = nc.dram_tensor("diag_l", [128, 32 * 128], mdt, kind="ExternalInput")
    d_dr = nc.dram_tensor("diag_r", [128, 32 * 128], mdt, kind="ExternalInput")
    d_i128 = nc.dram_tensor("i128", [128, 128], mdt, kind="ExternalInput")
    d_i64p = nc.dram_tensor("i64p", [64, 128], mdt, kind="ExternalInput")

    slab_m = nc.dram_tensor("slab_m", [RB, 384, CH], f32, kind="ExternalOutput")
    out0d = nc.dram_tensor("out0d", [32, 128, CH], f32, kind="ExternalOutput")
    out0u = nc.dram_tensor("out0u", [32, 128, CH], f32, kind="ExternalOutput")

    # flat per-row scratch (A / B' tables flattened row-major) in DRAM.
    # One tensor so the SBUF flat-table tile has only 2 producer DMAs
    # (matmul wait-slot limit).
    nscr = 4 if mode == "bf16hl" else 2
    scr_all = nc.dram_tensor("scr_all", [nscr, npad, CH],
                             BF16 if mode == "bf16hl" else f32)

    def mm(out_ap, lhsT, rhs, start, stop):
        if mode == "f32r":
            lhsT = lhsT.bitcast(F32R)
            rhs = rhs.bitcast(F32R)
        nc.tensor.matmul(out_ap, lhsT, rhs, start=start, stop=stop)

    with tile.TileContext(nc) as tc:
        with (
            tc.tile_pool(name="const", bufs=1) as cpool,
            tc.tile_pool(name="tmp", bufs=3) as tpool,
            tc.tile_pool(name="psA", bufs=4, space="PSUM") as psA,
            tc.tile_pool(name="ps0", bufs=4, space="PSUM") as ps0,
            tc.tile_pool(name="stM", bufs=4) as stM,
            tc.tile_pool(name="st0", bufs=4) as st0,
        ):
            # ---- load inputs ---------------------------------------------
            def load(dram, shape, dtype, tag):
                t = cpool.tile(shape, dtype, tag=tag)
                nc.sync.dma_start(out=t[:], in_=dram[:])
                return t

            xt0 = load(xt_rot[0:128, :], [128, N], f32, "xt0")
            xt1 = load(xt_rot[128:256, :], [128, N], f32, "xt1")
            w1a = load(w1t[0:128, :], [128, CH], f32, "w1a")
            w1b = load(w1t[128:256, :], [128, CH], f32, "w1b")
            w2a = load(w2t[0:128, :], [128, CH], f32, "w2a")
            w2b = load(w2t[128:256, :], [128, CH], f32, "w2b")
            bt = load(b_row, [1, CH], f32, "bt")
            cmt = load(cm, [128, 8], f32, "cmt")
            wmt = load(d_wm, list(d_wm.shape), mdt, "wmt")
            wm0t = load(d_wm0, list(d_wm0.shape), mdt, "wm0t")
            dlt = load(d_dl, [128, 32 * 128], mdt, "dlt")
            drt = load(d_dr, [128, 32 * 128], mdt, "drt")
            i128t = load(d_i128, [128, 128], mdt, "i128t")
            i64pt = load(d_i64p, [64, 128], mdt, "i64pt")

            ones1 = cpool.tile([1, 128], f32, tag="ones1")
            nc.vector.memset(ones1[:], 1.0)

            # ---- phase 1: build tables A, B' (f32, exact) ----------------
            A_t, Bp_t = [], []
            for s in range(4):
                pa = ps0.tile([128, CH], f32, tag="p0")
                mmd = nc.tensor.matmul
                mmd(pa[:], xt0[:, 128 * s:128 * (s + 1)], w1a[:],
                    start=True, stop=False)
                mmd(pa[:], xt1[:, 128 * s:128 * (s + 1)], w1b[:],
                    start=False, stop=True)
                at = cpool.tile([128, CH], f32, tag=f"A{s}")
                nc.vector.tensor_copy(out=at[:], in_=pa[:])
                A_t.append(at)

                pb = ps0.tile([128, CH], f32, tag="p0")
                mmd(pb[:], xt0[:, 128 * s:128 * (s + 1)], w2a[:],
                    start=True, stop=False)
                mmd(pb[:], xt1[:, 128 * s:128 * (s + 1)], w2b[:],
                    start=False, stop=False)
                mmd(pb[:], ones1[:], bt[:], start=False, stop=True)
                bpt = cpool.tile([128, CH], f32, tag=f"B{s}")
                nc.scalar.copy(out=bpt[:], in_=pb[:])
                Bp_t.append(bpt)

            # ---- phase 1b: mixed column tables ---------------------------
            # Cmix[s] = R*B' + L*A   (f32)
            Cmix = []
            for s in range(4):
                t1 = tpool.tile([128, CH], f32, tag="t1")
                nc.vector.tensor_scalar_mul(t1[:], Bp_t[s][:], cmt[:, s:s + 1])
                t2 = tpool.tile([128, CH], f32, tag="t2")
                nc.vector.tensor_scalar_mul(t2[:], A_t[s][:], cmt[:, 4 + s:5 + s])
                cx = cpool.tile([128, CH], f32, tag=f"C{s}")
                nc.vector.tensor_add(cx[:], t1[:], t2[:])
                Cmix.append(cx)

            def hi_lo(src_ap, tag):
                """split a f32 [128, W] AP into bf16 hi + lo tiles."""
                wdt = src_ap.shape[-1]
                hi = cpool.tile([128, wdt], BF16, tag=f"{tag}h")
                nc.vector.tensor_copy(out=hi[:], in_=src_ap)
                h32 = tpool.tile([128, wdt], f32, tag="h32")
                nc.vector.tensor_copy(out=h32[:], in_=hi[:])
                d = tpool.tile([128, wdt], f32, tag="d32")
                nc.vector.tensor_sub(d[:], src_ap, h32[:])
                lo = cpool.tile([128, wdt], BF16, tag=f"{tag}l")
                nc.vector.tensor_copy(out=lo[:], in_=d[:])
                return hi, lo

            # duplicated column tables for the r-paired main tiles,
            # upper-half table for block0, diag combined table
            if mode == "bf16hl":
                CD = {}
                C0 = {}
                for s in (1, 2, 3):
                    hi, lo = hi_lo(Cmix[s][:], f"cs{s}")
                    dh = cpool.tile([128, 2 * CH], BF16, tag=f"CDh{s}")
                    nc.vector.tensor_copy(out=dh[:, 0:CH], in_=hi[:])
                    nc.scalar.copy(out=dh[:, CH:2 * CH], in_=hi[:])
                    dl_ = cpool.tile([128, 2 * CH], BF16, tag=f"CDl{s}")
                    nc.vector.tensor_copy(out=dl_[:, 0:CH], in_=lo[:])
                    nc.scalar.copy(out=dl_[:, CH:2 * CH], in_=lo[:])
                    CD[s] = (dh, dl_)
                c0h, c0l = hi_lo(Cmix[0][:], "cs0")
                cuh = cpool.tile([64, CH], BF16, tag="cuh")
                cul = cpool.tile([64, CH], BF16, tag="cul")
                nc.sync.dma_start(out=cuh[:], in_=c0h[64:128, :])
                nc.sync.dma_start(out=cul[:], in_=c0l[64:128, :])
                C0 = (cuh, cul)
                ah, al = hi_lo(A_t[0][:], "a0")
                bh, bl = hi_lo(Bp_t[0][:], "b0")
                dcb_h = cpool.tile([128, CH], BF16, tag="dcbh")
                dcb_l = cpool.tile([128, CH], BF16, tag="dcbl")
                nc.vector.tensor_copy(out=dcb_h[0:64, :], in_=ah[0:64, :])
                nc.vector.tensor_copy(out=dcb_l[0:64, :], in_=al[0:64, :])
                nc.sync.dma_start(out=dcb_h[64:128, :], in_=bh[0:64, :])
                nc.sync.dma_start(out=dcb_l[64:128, :], in_=bl[0:64, :])
                # flat scratch -> RP (K-contiguous row tables)
                nc.sync.dma_start(out=scr_all[0, 0:64, :], in_=ah[0:64, :])
                nc.sync.dma_start(out=scr_all[1, 0:64, :], in_=bh[0:64, :])
                nc.sync.dma_start(out=scr_all[2, 0:64, :], in_=al[0:64, :])
                nc.sync.dma_start(out=scr_all[3, 0:64, :], in_=bl[0:64, :])
                rp4 = cpool.tile([8, 64 * CH], BF16, tag="rp4")
                nc.sync.dma_start(
                    out=rp4[0:4, :],
                    in_=scr_all[:, 0:64, :].rearrange("c r ch -> c (r ch)"))
                nc.sync.dma_start(
                    out=rp4[4:8, 0:63 * CH],
                    in_=scr_all[:, 1:64, :].rearrange("c r ch -> c (r ch)"))
            else:
                CD = {}
                for s in (1, 2, 3):
                    dup = cpool.tile([128, 2 * CH], f32, tag=f"CD{s}")
                    nc.vector.tensor_copy(out=dup[:, 0:CH], in_=Cmix[s][:])
                    nc.scalar.copy(out=dup[:, CH:2 * CH], in_=Cmix[s][:])
                    CD[s] = dup
                cup = cpool.tile([64, CH], f32, tag="cup")
                nc.sync.dma_start(out=cup[:], in_=Cmix[0][64:128, :])
                dcb = cpool.tile([128, CH], f32, tag="dcb")
                nc.vector.tensor_copy(out=dcb[0:64, :], in_=A_t[0][0:64, :])
                nc.sync.dma_start(out=dcb[64:128, :], in_=Bp_t[0][0:64, :])
                nc.sync.dma_start(out=scr_all[0, 0:64, :], in_=A_t[0][0:64, :])
                nc.sync.dma_start(out=scr_all[1, 0:64, :], in_=Bp_t[0][0:64, :])
                rp4 = cpool.tile([4, 64 * CH], f32, tag="rp4")
                nc.sync.dma_start(
                    out=rp4[0:2, :],
                    in_=scr_all[:, 0:64, :].rearrange("c r ch -> c (r ch)"))
                nc.sync.dma_start(
                    out=rp4[2:4, 0:63 * CH],
                    in_=scr_all[:, 1:64, :].rearrange("c r ch -> c (r ch)"))

            # ---- phase 2: main loop --------------------------------------
            cp_i = 0

            def cp(out_ap, in_ap):
                nonlocal cp_i
                if cp_i % 2 == 0:
                    nc.vector.tensor_copy(out=out_ap, in_=in_ap)
                else:
                    nc.scalar.copy(out=out_ap, in_=in_ap)
                cp_i += 1

            for g in range(8):
                sM = {J: stM.tile([128, 4 * 512], f32, tag="sm",
                                  name=f"sm_{g}_{J}")
                      for J in (1, 2, 3)}
                s0d = st0.tile([128, 4 * CH], f32, tag="s0")
                s0u = st0.tile([128, 4 * CH], f32, tag="s0")
                for sub in range(4):
                    rp = 4 * g + sub
                    off = 2 * rp * CH
                    # main 128-wide column blocks J = 1..3 (s in [128,512))
                    for J in (1, 2, 3):
                        p = psA.tile([128, 512], f32, tag="pj")
                        if mode == "bf16hl":
                            dh, dl_ = CD[J]
                            mm(p[:], i128t[:], dh[:], True, False)
                            mm(p[:], i128t[:], dl_[:], False, False)
                            mm(p[:], wmt[0:4, 128 * J:128 * (J + 1)],
                               rp4[0:4, off:off + 512], False, True)
                        else:
                            mm(p[:], i128t[:], CD[J][:], True, False)
                            mm(p[:], wmt[0:2, 128 * J:128 * (J + 1)],
                               rp4[0:2, off:off + 512], False, True)
                        cp(sM[J][:, 512 * sub:512 * (sub + 1)], p[:])
                    # block 0 upper half (s in [64,128)), rows r0, r0+1
                    pu = ps0.tile([128, CH], f32, tag="p0")
                    if mode == "bf16hl":
                        mm(pu[:], i64pt[:], C0[0][:], True, False)
                        mm(pu[:], i64pt[:], C0[1][:], False, False)
                        mm(pu[:], wm0t[:], rp4[0:8, off:off + CH], False, True)
                    else:
                        mm(pu[:], i64pt[:], cup[:], True, False)
                        mm(pu[:], wm0t[:], rp4[0:4, off:off + CH], False, True)
                    cp(s0u[:, CH * sub:CH * (sub + 1)], pu[:])
                    # diagonal block (s in [0,64)), rows r0, r0+1
                    pd = ps0.tile([128, CH], f32, tag="p0")
                    dl_sl = dlt[:, 128 * rp:128 * (rp + 1)]
                    dr_sl = drt[:, 128 * rp:128 * (rp + 1)]
                    if mode == "bf16hl":
                        mm(pd[:], dl_sl, dcb_h[:], True, False)
                        mm(pd[:], dl_sl, dcb_l[:], False, False)
                        mm(pd[:], dr_sl, dcb_h[:], False, False)
                        mm(pd[:], dr_sl, dcb_l[:], False, True)
                    else:
                        mm(pd[:], dl_sl, dcb[:], True, False)
                        mm(pd[:], dr_sl, dcb[:], False, True)
                    cp(s0d[:, CH * sub:CH * (sub + 1)], pd[:])

                # group DMAs: 8 output rows (4 sub-pairs) per tensor
                for J in (1, 2, 3):
                    dest = slab_m[8 * g:8 * (g + 1),
                                  128 * (J - 1):128 * J, :]
                    dest = dest.rearrange("(sub q) p c -> p sub q c", q=2)
                    src = sM[J][:].rearrange("p (sub q c) -> p sub q c",
                                             sub=4, q=2)
                    nc.sync.dma_start(out=dest, in_=src)
                nc.sync.dma_start(
                    out=out0u[4 * g:4 * (g + 1)].rearrange("s p c -> p s c"),
                    in_=s0u[:].rearrange("p (s c) -> p s c", s=4))
                nc.sync.dma_start(
                    out=out0d[4 * g:4 * (g + 1)].rearrange("s p c -> p s c"),
                    in_=s0d[:].rearrange("p (s c) -> p s c", s=4))
    nc.compile()
    return nc


def _program(mode: str) -> bass.Bass:
    if mode not in _PROGRAMS:
        _PROGRAMS[mode] = _build_program(mode)
    return _PROGRAMS[mode]


# --------------------------------------------------------------------------
# host entry point
# --------------------------------------------------------------------------

def _assemble(results):
    """8 per-core result dicts -> full [512, 512, 256] output."""
    out = np.empty((N, N, CH), np.float32)
    for k in range(NCORES):
        r = results[k]
        slab = np.empty((RB, N, CH), np.float32)
        slab[:, 0:64, :] = np.asarray(r["out0d"]).reshape(RB, 64, CH)
        slab[:, 64:128, :] = np.asarray(r["out0u"]).reshape(RB, 64, CH)
        slab[:, 128:512, :] = np.asarray(r["slab_m"])
        base = RB * k
        out[base:base + RB] = np.roll(slab, base, axis=1)
    return out


def build_in_maps(x, W, b, mode=None):
    mode = mode or MODE
    shared = _shared_inputs(W, b, mode)
    return [dict(shared, **_core_inputs(x, k, mode)) for k in range(NCORES)]


def kernel(x, W, b):
    nc = _program(MODE)
    in_maps = build_in_maps(x, W, b, MODE)
    res = run_bass_kernel_spmd(nc, in_maps, core_ids=list(range(NCORES)))
    return _assemble(res.results)


# revision 12
# speedup vs baseline: 1.5686x; 1.5686x over previous
"""Trainium2 Bass kernel for nn_LinearEncoder (gnn_message_passing).

Reference computes, for N=512 nodes with n_in = n_out = 256:
    i, j = triu_indices(N, k=1)
    edges = concat([x[i], x[j]], -1)            # [E, 512]
    h = edges @ W.T + b                         # [E, 256]
    out[i, j] = h ; out = out + out.T           # [N, N, 256], 0 diagonal

Key algebraic identity: with W = [W1 | W2],
    h(i, j) = A[i] + B[j] + b,   A = x @ W1.T,  B = x @ W2.T
so the full output is
    out[i, j] = A[min(i,j)] + B'[max(i,j)]      (B' = B + b), 0 on diagonal.

The kernel shards output rows across 8 cores (64 rows each).  To keep a
single SPMD program, core k receives x pre-rotated by its row base
(x_rot[t] = x[(base+t) % 512]), which puts the triangular "diagonal block"
at local columns s in [0, 64) for every core.  Column/row table selection
(A vs B') differs per core only through small 0/1 mask *inputs*.  All the
broadcast-adds run on the TensorEngine as PSUM-accumulated matmuls:

    psum  = I128.T   @ ColTable          (copies the column table)
    psum += masks.T  @ row-table-slice   (adds the broadcast row term)

and the triangular diagonal block (including its zero diagonal) comes from
two constant masked-selection matmuls.  ScalarE/VectorE only evacuate PSUM
to SBUF; DMA streams ~33.5 MB/core of output back to HBM (the roofline).
"""

import os
import sys

for _p in ("/opt/trn_rl_repo", "/root/.axon_site/_ro/trn_rl_repo"):
    if os.path.isdir(_p) and _p not in sys.path:
        sys.path.insert(0, _p)

import numpy as np
import ml_dtypes

import concourse.bass as bass
import concourse.bacc as bacc
import concourse.mybir as mybir
import concourse.tile as tile
from concourse.bass_utils import run_bass_kernel_spmd

N = 512
CH = 256          # n_out
NIN = 256         # n_in
NCORES = 8
RB = N // NCORES  # 64 rows per core
F32 = mybir.dt.float32
F32R = mybir.dt.float32r
BF16 = mybir.dt.bfloat16
BF16NP = ml_dtypes.bfloat16

MODE = os.environ.get("LK_MODE", "bf16hl")  # "f32" | "f32r" | "bf16hl"


# --------------------------------------------------------------------------
# host-side constant builders
# --------------------------------------------------------------------------

def _masks_RL(k: int):
    """R/L region indicators over local columns s for core k."""
    base = RB * k
    wrap = N - base  # columns s >= wrap hold wrapped (j < base) entries
    s = np.arange(N)
    R = ((s >= 64) & (s < wrap)).astype(np.float32)
    L = (s >= wrap).astype(np.float32)
    return R, L


def _diag_consts():
    """Constant masked-selection weights for the 64x64 diagonal blocks.

    For row-pair rp, output column m = q*64 + s (q in {0,1}, s in [0,64)),
    with r_q = 2*rp + q and rhs = [A_rot[0:64] ; B'_rot[0:64]] (K = 128):
      L side (s < r_q):  value = A_rot[s] + B'_rot[r_q]
      R side (s > r_q):  value = B'_rot[s] + A_rot[r_q]
      s == r_q: all weights zero -> exact 0 output.
    """
    dl = np.zeros((128, 32 * 128), np.float32)
    dr = np.zeros((128, 32 * 128), np.float32)
    for rp in range(32):
        for q in range(2):
            r_q = 2 * rp + q
            for s in range(64):
                m = rp * 128 + q * 64 + s
                if s < r_q:
                    dl[s, m] = 1.0            # A_rot[s]
                    dl[64 + r_q, m] = 1.0     # B'_rot[r_q]
                elif s > r_q:
                    dr[64 + s, m] = 1.0       # B'_rot[s]
                    dr[r_q, m] = 1.0          # A_rot[r_q]
    return dl, dr


def _shared_inputs(W: np.ndarray, b: np.ndarray, mode: str):
    W = np.asarray(W, np.float32)
    b = np.asarray(b, np.float32)
    dl, dr = _diag_consts()
    i128 = np.eye(128, dtype=np.float32)
    i64p = np.concatenate([np.eye(64, dtype=np.float32)] * 2, axis=1)  # [64,128]
    out = {
        "w1t": np.ascontiguousarray(W[:, :NIN].T),   # [in, out]
        "w2t": np.ascontiguousarray(W[:, NIN:].T),
        "b_row": b.reshape(1, CH).copy(),
    }
    if mode == "bf16hl":
        cast = lambda a: a.astype(BF16NP)
    else:
        cast = lambda a: a.astype(np.float32)
    out["diag_l"] = cast(dl)
    out["diag_r"] = cast(dr)
    out["i128"] = cast(i128)
    out["i64p"] = cast(i64p)
    return out


def _core_inputs(x: np.ndarray, k: int, mode: str):
    x = np.asarray(x, np.float32)
    base = RB * k
    x_rot = np.roll(x, -base, axis=0)
    R, L = _masks_RL(k)

    cm = np.zeros((128, 8), np.float32)
    for t in range(4):
        cm[:, t] = R[128 * t:128 * (t + 1)]
        cm[:, 4 + t] = L[128 * t:128 * (t + 1)]

    if mode == "bf16hl":
        wm = np.stack([R, L, R, L]).astype(BF16NP)          # [4, 512]
        wm0 = np.zeros((8, 128), np.float32)
        p = np.arange(64)
        wm0[0, :64] = R[64 + p]
        wm0[1, :64] = L[64 + p]
        wm0[2, :64] = R[64 + p]
        wm0[3, :64] = L[64 + p]
        wm0[4, 64:] = R[64 + p]
        wm0[5, 64:] = L[64 + p]
        wm0[6, 64:] = R[64 + p]
        wm0[7, 64:] = L[64 + p]
        wm0 = wm0.astype(BF16NP)
    else:
        wm = np.stack([R, L]).astype(np.float32)            # [2, 512]
        wm0 = np.zeros((4, 128), np.float32)
        p = np.arange(64)
        wm0[0, :64] = R[64 + p]
        wm0[1, :64] = L[64 + p]
        wm0[2, 64:] = R[64 + p]
        wm0[3, 64:] = L[64 + p]
    return {
        "xt_rot": np.ascontiguousarray(x_rot.T),  # [in=256, node=512]
        "cm": cm,
        "wm": wm,
        "wm0": wm0,
    }


# --------------------------------------------------------------------------
# device program
# --------------------------------------------------------------------------

_PROGRAMS: dict = {}


def _build_program(mode: str) -> bass.Bass:
    nc = bacc.Bacc()
    f32 = F32
    mdt = {"f32": F32, "f32r": F32R, "bf16hl": BF16}[mode]
    tdt = F32R if mode == "f32r" else F32  # device-built matmul operand tables
    npad = 68  # padded flat scratch rows

    # ---- dram tensors -----------------------------------------------------
    xt_rot = nc.dram_tensor("xt_rot", [NIN, N], f32, kind="ExternalInput")
    w1t = nc.dram_tensor("w1t", [NIN, CH], f32, kind="ExternalInput")
    w2t = nc.dram_tensor("w2t", [NIN, CH], f32, kind="ExternalInput")
    b_row = nc.dram_tensor("b_row", [1, CH], f32, kind="ExternalInput")
    cm = nc.dram_tensor("cm", [128, 8], f32, kind="ExternalInput")
    d_wm = nc.dram_tensor("wm", [4 if mode == "bf16hl" else 2, N], mdt,
                          kind="ExternalInput")
    d_wm0 = nc.dram_tensor("wm0", [8 if mode == "bf16hl" else 4, 128], mdt,
                           kind="ExternalInput")
    d_dl = nc.dram_tensor("diag_l", [128, 32 * 128], mdt, kind="ExternalInput")
    d_dr = nc.dram_tensor("diag_r", [128, 32 * 128], mdt, kind="ExternalInput")
    d_i128 = nc.dram_tensor("i128", [128, 128], mdt, kind="ExternalInput")
    d_i64p = nc.dram_tensor("i64p", [64, 128], mdt, kind="ExternalInput")

    slab_m = nc.dram_tensor("slab_m", [RB, 384, CH], f32, kind="ExternalOutput")
    out0d = nc.dram_tensor("out0d", [32, 128, CH], f32, kind="ExternalOutput")
    out0u = nc.dram_tensor("out0u", [32, 128, CH], f32, kind="ExternalOutput")

    # flat per-row scratch (A / B' tables flattened row-major) in DRAM.
    # One tensor so the SBUF flat-table tile has only 2 producer DMAs
    # (matmul wait-slot limit).
    nscr = 4 if mode == "bf16hl" else 2
    scr_all = nc.dram_tensor("scr_all", [nscr, npad, CH],
                             BF16 if mode == "bf16hl" else tdt)

    def mm(out_ap, lhsT, rhs, start, stop):
        nc.tensor.matmul(out_ap, lhsT, rhs, start=start, stop=stop)

    with tile.TileContext(nc) as tc:
        with (
            tc.tile_pool(name="const", bufs=1) as cpool,
            tc.tile_pool(name="tmp", bufs=3) as tpool,
            tc.tile_pool(name="psA", bufs=4, space="PSUM") as psA,
            tc.tile_pool(name="ps0", bufs=4, space="PSUM") as ps0,
            tc.tile_pool(name="stM", bufs=6) as stM,
            tc.tile_pool(name="st0", bufs=4) as st0,
        ):
            # ---- load inputs ---------------------------------------------
            def load(dram, shape, dtype, tag):
                t = cpool.tile(shape, dtype, tag=tag)
                nc.sync.dma_start(out=t[:], in_=dram[:])
                return t

            xt0 = load(xt_rot[0:128, :], [128, N], f32, "xt0")
            xt1 = load(xt_rot[128:256, :], [128, N], f32, "xt1")
            w1a = load(w1t[0:128, :], [128, CH], f32, "w1a")
            w1b = load(w1t[128:256, :], [128, CH], f32, "w1b")
            w2a = load(w2t[0:128, :], [128, CH], f32, "w2a")
            w2b = load(w2t[128:256, :], [128, CH], f32, "w2b")
            bt = load(b_row, [1, CH], f32, "bt")
            cmt = load(cm, [128, 8], f32, "cmt")
            wmt = load(d_wm, list(d_wm.shape), mdt, "wmt")
            wm0t = load(d_wm0, list(d_wm0.shape), mdt, "wm0t")
            dlt = load(d_dl, [128, 32 * 128], mdt, "dlt")
            drt = load(d_dr, [128, 32 * 128], mdt, "drt")
            i128t = load(d_i128, [128, 128], mdt, "i128t")
            i64pt = load(d_i64p, [64, 128], mdt, "i64pt")

            ones1 = cpool.tile([1, 128], f32, tag="ones1")
            nc.vector.memset(ones1[:], 1.0)

            # ---- phase 1: build tables A, B' (f32, exact) ----------------
            A_t, Bp_t = [], []
            for s in range(4):
                pa = ps0.tile([128, CH], f32, tag="p0")
                mmd = nc.tensor.matmul
                mmd(pa[:], xt0[:, 128 * s:128 * (s + 1)], w1a[:],
                    start=True, stop=False)
                mmd(pa[:], xt1[:, 128 * s:128 * (s + 1)], w1b[:],
                    start=False, stop=True)
                at = cpool.tile([128, CH], tdt, tag=f"A{s}")
                nc.vector.tensor_copy(out=at[:], in_=pa[:])
                A_t.append(at)

                pb = ps0.tile([128, CH], f32, tag="p0")
                mmd(pb[:], xt0[:, 128 * s:128 * (s + 1)], w2a[:],
                    start=True, stop=False)
                mmd(pb[:], xt1[:, 128 * s:128 * (s + 1)], w2b[:],
                    start=False, stop=False)
                mmd(pb[:], ones1[:], bt[:], start=False, stop=True)
                bpt = cpool.tile([128, CH], tdt, tag=f"B{s}")
                nc.scalar.copy(out=bpt[:], in_=pb[:])
                Bp_t.append(bpt)

            # ---- phase 1b: mixed column tables ---------------------------
            # Cmix[s] = R*B' + L*A   (f32)
            Cmix = []
            for s in range(4):
                t1 = tpool.tile([128, CH], f32, tag="t1")
                nc.vector.tensor_scalar_mul(t1[:], Bp_t[s][:], cmt[:, s:s + 1])
                t2 = tpool.tile([128, CH], f32, tag="t2")
                nc.vector.tensor_scalar_mul(t2[:], A_t[s][:], cmt[:, 4 + s:5 + s])
                cx = cpool.tile([128, CH], tdt, tag=f"C{s}")
                nc.vector.tensor_add(cx[:], t1[:], t2[:])
                Cmix.append(cx)

            def hi_lo(src_ap, tag):
                """split a f32 [128, W] AP into bf16 hi + lo tiles."""
                wdt = src_ap.shape[-1]
                hi = cpool.tile([128, wdt], BF16, tag=f"{tag}h")
                nc.vector.tensor_copy(out=hi[:], in_=src_ap)
                h32 = tpool.tile([128, wdt], f32, tag="h32")
                nc.vector.tensor_copy(out=h32[:], in_=hi[:])
                d = tpool.tile([128, wdt], f32, tag="d32")
                nc.vector.tensor_sub(d[:], src_ap, h32[:])
                lo = cpool.tile([128, wdt], BF16, tag=f"{tag}l")
                nc.vector.tensor_copy(out=lo[:], in_=d[:])
                return hi, lo

            # duplicated column tables for the r-paired main tiles,
            # upper-half table for block0, diag combined table
            if mode == "bf16hl":
                # f32 duplicated column tables — folded in during PSUM
                # evacuation (DVE tensor_add), so they stay exact.
                CD = {}
                for s in (1, 2, 3):
                    dup = cpool.tile([128, 2 * CH], f32, tag=f"CD{s}")
                    nc.vector.tensor_copy(out=dup[:, 0:CH], in_=Cmix[s][:])
                    nc.scalar.copy(out=dup[:, CH:2 * CH], in_=Cmix[s][:])
                    CD[s] = dup
                c0h, c0l = hi_lo(Cmix[0][:], "cs0")
                cuh = cpool.tile([64, CH], BF16, tag="cuh")
                cul = cpool.tile([64, CH], BF16, tag="cul")
                nc.sync.dma_start(out=cuh[:], in_=c0h[64:128, :])
                nc.sync.dma_start(out=cul[:], in_=c0l[64:128, :])
                C0 = (cuh, cul)
                ah, al = hi_lo(A_t[0][:], "a0")
                bh, bl = hi_lo(Bp_t[0][:], "b0")
                dcb_h = cpool.tile([128, CH], BF16, tag="dcbh")
                dcb_l = cpool.tile([128, CH], BF16, tag="dcbl")
                nc.vector.tensor_copy(out=dcb_h[0:64, :], in_=ah[0:64, :])
                nc.vector.tensor_copy(out=dcb_l[0:64, :], in_=al[0:64, :])
                nc.sync.dma_start(out=dcb_h[64:128, :], in_=bh[0:64, :])
                nc.sync.dma_start(out=dcb_l[64:128, :], in_=bl[0:64, :])
                # flat scratch -> RP (K-contiguous row tables)
                nc.sync.dma_start(out=scr_all[0, 0:64, :], in_=ah[0:64, :])
                nc.sync.dma_start(out=scr_all[1, 0:64, :], in_=bh[0:64, :])
                nc.sync.dma_start(out=scr_all[2, 0:64, :], in_=al[0:64, :])
                nc.sync.dma_start(out=scr_all[3, 0:64, :], in_=bl[0:64, :])
                rp4 = cpool.tile([8, 64 * CH], BF16, tag="rp4")
                nc.sync.dma_start(
                    out=rp4[0:4, :],
                    in_=scr_all[:, 0:64, :].rearrange("c r ch -> c (r ch)"))
                nc.sync.dma_start(
                    out=rp4[4:8, 0:63 * CH],
                    in_=scr_all[:, 1:64, :].rearrange("c r ch -> c (r ch)"))
            else:
                CD = {}
                for s in (1, 2, 3):
                    dup = cpool.tile([128, 2 * CH], tdt, tag=f"CD{s}")
                    nc.vector.tensor_copy(out=dup[:, 0:CH], in_=Cmix[s][:])
                    nc.scalar.copy(out=dup[:, CH:2 * CH], in_=Cmix[s][:])
                    CD[s] = dup
                cup = cpool.tile([64, CH], tdt, tag="cup")
                nc.sync.dma_start(out=cup[:], in_=Cmix[0][64:128, :])
                dcb = cpool.tile([128, CH], tdt, tag="dcb")
                nc.vector.tensor_copy(out=dcb[0:64, :], in_=A_t[0][0:64, :])
                nc.sync.dma_start(out=dcb[64:128, :], in_=Bp_t[0][0:64, :])
                nc.sync.dma_start(out=scr_all[0, 0:64, :], in_=A_t[0][0:64, :])
                nc.sync.dma_start(out=scr_all[1, 0:64, :], in_=Bp_t[0][0:64, :])
                rp4 = cpool.tile([4, 64 * CH], tdt, tag="rp4")
                nc.sync.dma_start(
                    out=rp4[0:2, :],
                    in_=scr_all[:, 0:64, :].rearrange("c r ch -> c (r ch)"))
                nc.sync.dma_start(
                    out=rp4[2:4, 0:63 * CH],
                    in_=scr_all[:, 1:64, :].rearrange("c r ch -> c (r ch)"))

            # ---- phase 2: main loop --------------------------------------
            cp_i = 0

            def cp(out_ap, in_ap):
                nonlocal cp_i
                if cp_i % 2 == 0:
                    nc.vector.tensor_copy(out=out_ap, in_=in_ap)
                else:
                    nc.scalar.copy(out=out_ap, in_=in_ap)
                cp_i += 1

            for g in range(8):
                sM = {J: stM.tile([128, 4 * 512], f32, tag="sm",
                                  name=f"sm_{g}_{J}")
                      for J in (1, 2, 3)}
                s0d = st0.tile([128, 4 * CH], f32, tag="s0")
                s0u = st0.tile([128, 4 * CH], f32, tag="s0")
                for sub in range(4):
                    rp = 4 * g + sub
                    off = 2 * rp * CH
                    # main 128-wide column blocks J = 1..3 (s in [128,512))
                    for J in (1, 2, 3):
                        p = psA.tile([128, 512], f32, tag="pj")
                        if mode == "bf16hl":
                            mm(p[:], wmt[0:4, 128 * J:128 * (J + 1)],
                               rp4[0:4, off:off + 512], True, True)
                            nc.vector.tensor_add(
                                sM[J][:, 512 * sub:512 * (sub + 1)],
                                p[:], CD[J][:])
                        else:
                            mm(p[:], i128t[:], CD[J][:], True, False)
                            mm(p[:], wmt[0:2, 128 * J:128 * (J + 1)],
                               rp4[0:2, off:off + 512], False, True)
                            cp(sM[J][:, 512 * sub:512 * (sub + 1)], p[:])
                    # block 0 upper half (s in [64,128)), rows r0, r0+1
                    pu = ps0.tile([128, CH], f32, tag="p0")
                    if mode == "bf16hl":
                        mm(pu[:], i64pt[:], C0[0][:], True, False)
                        mm(pu[:], i64pt[:], C0[1][:], False, False)
                        mm(pu[:], wm0t[:], rp4[0:8, off:off + CH], False, True)
                    else:
                        mm(pu[:], i64pt[:], cup[:], True, False)
                        mm(pu[:], wm0t[:], rp4[0:4, off:off + CH], False, True)
                    if mode == "bf16hl":
                        nc.scalar.copy(out=s0u[:, CH * sub:CH * (sub + 1)],
                                       in_=pu[:])
                    else:
                        cp(s0u[:, CH * sub:CH * (sub + 1)], pu[:])
                    # diagonal block (s in [0,64)), rows r0, r0+1
                    pd = ps0.tile([128, CH], f32, tag="p0")
                    dl_sl = dlt[:, 128 * rp:128 * (rp + 1)]
                    dr_sl = drt[:, 128 * rp:128 * (rp + 1)]
                    if mode == "bf16hl":
                        mm(pd[:], dl_sl, dcb_h[:], True, False)
                        mm(pd[:], dl_sl, dcb_l[:], False, False)
                        mm(pd[:], dr_sl, dcb_h[:], False, False)
                        mm(pd[:], dr_sl, dcb_l[:], False, True)
                    else:
                        mm(pd[:], dl_sl, dcb[:], True, False)
                        mm(pd[:], dr_sl, dcb[:], False, True)
                    if mode == "bf16hl":
                        nc.scalar.copy(out=s0d[:, CH * sub:CH * (sub + 1)],
                                       in_=pd[:])
                    else:
                        cp(s0d[:, CH * sub:CH * (sub + 1)], pd[:])

                # group DMAs: 8 output rows (4 sub-pairs) per tensor
                for J in (1, 2, 3):
                    dest = slab_m[8 * g:8 * (g + 1),
                                  128 * (J - 1):128 * J, :]
                    dest = dest.rearrange("(sub q) p c -> p sub q c", q=2)
                    src = sM[J][:].rearrange("p (sub q c) -> p sub q c",
                                             sub=4, q=2)
                    nc.sync.dma_start(out=dest, in_=src)
                nc.scalar.dma_start(
                    out=out0u[4 * g:4 * (g + 1)].rearrange("s p c -> p s c"),
                    in_=s0u[:].rearrange("p (s c) -> p s c", s=4))
                nc.scalar.dma_start(
                    out=out0d[4 * g:4 * (g + 1)].rearrange("s p c -> p s c"),
                    in_=s0d[:].rearrange("p (s c) -> p s c", s=4))
    nc.compile()
    return nc


def _program(mode: str) -> bass.Bass:
    if mode not in _PROGRAMS:
        _PROGRAMS[mode] = _build_program(mode)
    return _PROGRAMS[mode]


# --------------------------------------------------------------------------
# host entry point
# --------------------------------------------------------------------------

def _assemble(results):
    """8 per-core result dicts -> full [512, 512, 256] output."""
    out = np.empty((N, N, CH), np.float32)
    for k in range(NCORES):
        r = results[k]
        slab = np.empty((RB, N, CH), np.float32)
        slab[:, 0:64, :] = np.asarray(r["out0d"]).reshape(RB, 64, CH)
        slab[:, 64:128, :] = np.asarray(r["out0u"]).reshape(RB, 64, CH)
        slab[:, 128:512, :] = np.asarray(r["slab_m"])
        base = RB * k
        out[base:base + RB] = np.roll(slab, base, axis=1)
    return out


def build_in_maps(x, W, b, mode=None):
    mode = mode or MODE
    shared = _shared_inputs(W, b, mode)
    return [dict(shared, **_core_inputs(x, k, mode)) for k in range(NCORES)]


def kernel(x, W, b):
    nc = _program(MODE)
    in_maps = build_in_maps(x, W, b, MODE)
    res = run_bass_kernel_spmd(nc, in_maps, core_ids=list(range(NCORES)))
    return _assemble(res.results)
